# revision 25
# baseline (speedup 1.0000x reference)
"""Multi-head attention (N=2, L=2048, 16 heads x 64) on 8 TRN2 NeuronCores.

Sharding: batch x head hybrid. Cores 0-3 take batch 0, cores 4-7 batch 1;
within each 4-core group a core computes 4 heads (256 depth). Attention is
head-parallel; two small AllToAlls (one per 1024-row window, within the
4-core group) switch to sequence-parallel for the output projection,
pipelined so only the last one is exposed at the end.

Orientation: scores are computed transposed ([k, q]) so attention weights
feed the AV matmul as the moving operand. Heads are processed in pairs:
each score psum page holds 2 heads ([128, 2x512]) so exp is one ScalarE
instruction per page for full blocks; AV packs 2 heads per wave via
col-group tiling (out partitions 0-63 / 64-127) and the softmax
denominators of all 4 heads are rank-1 ones matmuls col-packed into one
psum tile (partitions 0/32/64/96).
"""
import sys

sys.path.insert(0, "/opt/trn_rl_repo")

import numpy as np
import ml_dtypes

import concourse.bass as bass
import concourse.bacc as bacc
import concourse.mybir as mybir
import concourse.tile as tile
from concourse.bass_utils import run_bass_kernel_spmd

BF16 = ml_dtypes.bfloat16

DM = 1024      # dmodel
DK = 64        # head dim
H = 16         # heads
NB = 2         # batch
L = 2048       # seq len per batch
NC = 8         # cores
NG = 4         # cores per group (one batch per group)
HPC = 4        # heads per core
DPC = HPC * DK  # depth per core = 256
VW = DPC       # v width per k-tile

SW = 512       # sub-window (qs granularity)
WW = 1024      # window (A2A granularity)
KT = 128       # k tile
NSW = L // SW   # 4 sub-windows
NWW = L // WW   # 2 windows
NKT = L // KT   # 16 k tiles
SROWS = WW // NC  # 128 rows per A2A shard
CROWS = 2 * SROWS  # 256 output rows per (core, window): 128 from each batch

_CACHE = {}


def _classify_blocks(mask):
    """Per (qs, kt): 0=skip, 1=full, 2=partial (+ q-span, pattern)."""
    mask = np.asarray(mask, dtype=bool)
    cls = [[0] * NKT for _ in range(NSW)]
    span = [[None] * NKT for _ in range(NSW)]
    pat_ids = {}
    pats = []
    pat_idx = [[-1] * NKT for _ in range(NSW)]
    for qs in range(NSW):
        for kt in range(NKT):
            sub = mask[qs * SW:(qs + 1) * SW, kt * KT:(kt + 1) * KT]
            rows = np.nonzero(sub.any(axis=1))[0]
            if rows.size == 0:
                cls[qs][kt] = 0
            elif sub.all():
                cls[qs][kt] = 1
                span[qs][kt] = (0, SW)
            else:
                cls[qs][kt] = 2
                span[qs][kt] = (int(rows[0]), int(rows[-1]) + 1)
                pat = np.ascontiguousarray(sub.T).astype(BF16)  # [128 k, SW q]
                key = pat.tobytes()
                if key not in pat_ids:
                    pat_ids[key] = len(pats)
                    pats.append(pat)
                pat_idx[qs][kt] = pat_ids[key]
    # general-mask safety: the first included kt of each sub-window must cover
    # the full 512 columns (its start=True matmul clears PSUM has_written)
    for qs in range(NSW):
        for kt in range(NKT):
            if cls[qs][kt]:
                span[qs][kt] = (0, SW)
                break
    if not pats:
        pats.append(np.ones((KT, SW), dtype=BF16))
    return cls, span, pat_idx, np.stack(pats)


def _build(cls, span, pat_idx, n_pat):
    nc = bacc.Bacc("TRN2", target_bir_lowering=False, debug=False,
                   enable_asserts=False, num_devices=NC)
    f32, bf16 = mybir.dt.float32, mybir.dt.bfloat16

    xtb = nc.dram_tensor("xtb", [DM, L], bf16, kind="ExternalInput")
    ytb = nc.dram_tensor("ytb", [DM, L], bf16, kind="ExternalInput")
    wq = nc.dram_tensor("wq", [DM, DPC], bf16, kind="ExternalInput")
    wk = nc.dram_tensor("wk", [DM, DPC], bf16, kind="ExternalInput")
    wv = nc.dram_tensor("wv", [DM, DPC], bf16, kind="ExternalInput")
    wo = nc.dram_tensor("wo", [DM, DM], bf16, kind="ExternalInput")
    bqd = nc.dram_tensor("bq", [128, 2], f32, kind="ExternalInput")
    bkd = nc.dram_tensor("bk", [128, 2], f32, kind="ExternalInput")
    bv1 = nc.dram_tensor("bv1", [1, DPC], bf16, kind="ExternalInput")
    bod = nc.dram_tensor("bo", [DM, 1], f32, kind="ExternalInput")
    mpat = nc.dram_tensor("mpat", [n_pat, KT, SW], bf16, kind="ExternalInput")
    out_t = nc.dram_tensor("out_t", [DM, NWW * CROWS], f32, kind="ExternalOutput")

    def qs_kts(qs):
        return [(kt, *span[qs][kt]) for kt in range(NKT) if cls[qs][kt]]

    with tile.TileContext(nc) as tc:
        with (
            tc.tile_pool(name="const", bufs=1) as cst,
            tc.tile_pool(name="xy", bufs=16) as xy,
            tc.tile_pool(name="big", bufs=1) as big,
            tc.tile_pool(name="exp", bufs=10) as expp,
            tc.tile_pool(name="sm", bufs=6) as sm,
            tc.tile_pool(name="ht", bufs=2) as htp,
            tc.tile_pool(name="rh", bufs=2) as rhp,
            tc.tile_pool(name="osb", bufs=3) as osb,
            tc.tile_pool(name="scp", bufs=2, space="PSUM") as scp,
            tc.tile_pool(name="avp", bufs=4, space="PSUM") as avp,
            tc.tile_pool(name="dram", bufs=1, space="DRAM") as dram,
            tc.tile_pool(name="dscr", bufs=8, space="DRAM") as dscrp,
        ):
            # ---- constants to SBUF (DMA issue order matters: wq/wk first
            # for warmup + first projections, then the x/y halves stream in
            # load_half below; wv/wo/mpat are queued after the input halves
            # they precede in consumption order) ----
            wq_sb = cst.tile([128, 8 * DPC], bf16)
            wk_sb = cst.tile([128, 8 * DPC], bf16)
            wv_sb = cst.tile([128, 8 * VW], bf16)
            wo_sb = cst.tile([128, 8 * DM], bf16)
            for dt in range(8):
                nc.sync.dma_start(wq_sb[:, dt * DPC:(dt + 1) * DPC], wq[dt * 128:(dt + 1) * 128, :])
                nc.sync.dma_start(wk_sb[:, dt * DPC:(dt + 1) * DPC], wk[dt * 128:(dt + 1) * 128, :])
            bq_sb = cst.tile([128, 2], f32)
            bk_sb = cst.tile([128, 2], f32)
            nc.sync.dma_start(bq_sb[:], bqd[:])
            nc.sync.dma_start(bk_sb[:], bkd[:])
            bv1_sb = cst.tile([1, DPC], bf16)
            nc.sync.dma_start(bv1_sb[:], bv1[:])
            bo_sb = cst.tile([128, 8], f32)
            for mt in range(8):
                nc.sync.dma_start(bo_sb[:, mt:mt + 1], bod[mt * 128:(mt + 1) * 128, :])
            mpat_sb = cst.tile([KT, n_pat * SW], bf16)
            ones_row = cst.tile([1, 128], bf16)
            nc.vector.memset(ones_row[:], 1.0)
            ones_col = cst.tile([128, 1], bf16)
            nc.vector.memset(ones_col[:], 1.0)
            # rank-1 selector rows at partitions {0,32,64,96} (same partitions
            # as the den rows of heads 0-3): even-head rows select out
            # partitions 0-63, odd-head rows 64-127. Used to broadcast each
            # recip row across its head's partitions via a PE matmul.
            sel128 = cst.tile([128, 128], bf16)
            nc.vector.memset(sel128[:], 0.0)
            for h in range(4):
                lo = 0 if h % 2 == 0 else DK
                nc.vector.memset(sel128[32 * h:32 * h + 1, lo:lo + DK], 1.0)

            def load_weights_late():
                for dt in range(8):
                    nc.sync.dma_start(wv_sb[:, dt * VW:(dt + 1) * VW],
                                      wv[dt * 128:(dt + 1) * 128, :])
                for p in range(n_pat):
                    nc.sync.dma_start(mpat_sb[:, p * SW:(p + 1) * SW], mpat[p])

            def load_wo():
                for dt in range(8):
                    nc.sync.dma_start(wo_sb[:, dt * DM:(dt + 1) * DM],
                                      wo[dt * 128:(dt + 1) * 128, :])

            # start-of-kernel barrier: absorbs per-core launch skew while the
            # big input DMAs stream, so the window AllToAlls aren't skewed
            bar_in = dram.tile([1, 8], f32)
            bar_out = dram.tile([1, 8], f32)
            barrier_sb = cst.tile([1, 8], f32, tag="barrier_sb")
            nc.vector.memset(barrier_sb[:], 0.0)
            nc.sync.dma_start(bar_in[:], barrier_sb[:])
            nc.gpsimd.collective_compute(
                "AllReduce", mybir.AluOpType.add,
                replica_groups=[list(range(NC))],
                ins=[bar_in.opt()], outs=[bar_out.opt()])

            # PE warmup while inputs stream (HAM un-throttle)
            for i in range(16):
                wps = scp.tile([128, WW], f32, tag="scp", name=f"warm{i}")
                nc.tensor.matmul(wps[:, :SW], wq_sb[:, 0:128],
                                 wq_sb[:, 0:SW], start=True, stop=True)

            qT = big.tile([128, 2 * L], bf16)   # mh block * L + col
            kT = big.tile([128, 2 * L], bf16)
            vaug = big.tile([128, NKT * VW], bf16)

            xt_tiles = {}

            def load_half(src, hf, tag):
                # 512-col chunks, all dm-tiles' first chunks before second
                # chunks, so the s=0 projection page can start ~2x earlier
                ts = [xy.tile([128, WW], bf16, tag="xy", name=f"{tag}{hf}_{dt}")
                      for dt in range(8)]
                for s in range(2):
                    for dt in range(8):
                        nc.scalar.dma_start(
                            ts[dt][:, s * SW:(s + 1) * SW],
                            src[dt * 128:(dt + 1) * 128,
                                hf * WW + s * SW:hf * WW + (s + 1) * SW])
                xt_tiles[tag, hf] = ts

            def qk_page(hf, tag, wsb, bsb, dst, s):
                tiles = xt_tiles[tag, hf]
                pg = scp.tile([128, WW], f32, tag="scp", name=f"p{tag}{hf}{s}")
                for mh in range(2):
                    for dt in range(8):
                        nc.tensor.matmul(
                            pg[:, mh * SW:(mh + 1) * SW],
                            wsb[:, dt * DPC + mh * 128:dt * DPC + (mh + 1) * 128],
                            tiles[dt][:, s * SW:(s + 1) * SW],
                            start=(dt == 0), stop=(dt == 7))
                for mh in range(2):
                    col = mh * L + hf * WW + s * SW
                    nc.vector.tensor_scalar_add(
                        dst[:, col:col + SW],
                        pg[:, mh * SW:(mh + 1) * SW],
                        bsb[:, mh:mh + 1])

            def v_kt(hf, j):
                tiles = xt_tiles["y", hf]
                kti = hf * 8 + j
                psv = avp.tile([128, SW], f32, tag="avp", name=f"v{kti}")
                for dt in range(8):
                    nc.tensor.matmul(psv[:, :VW],
                                     tiles[dt][:, j * KT:(j + 1) * KT],
                                     wv_sb[:, dt * VW:(dt + 1) * VW],
                                     start=(dt == 0), stop=False)
                nc.tensor.matmul(psv[:, :VW], ones_row[:],
                                 bv1_sb[:], start=False, stop=True)
                nc.vector.tensor_copy(vaug[:, kti * VW:(kti + 1) * VW],
                                      psv[:, :VW])

            def attention_qs(qs, headT, interleave=()):
                """Software-pipelined kt loop: scores(kt_i+1) are issued
                before AV/den(kt_i) so the PE never waits on the exp of the
                block it is about to consume. `interleave` items (thunks of
                extra PE work) are injected between kt stages."""
                kts = qs_kts(qs)
                s = qs % 2
                avA = avp.tile([128, SW], f32, tag="avp", name=f"avA{qs}")
                avB = avp.tile([128, SW], f32, tag="avp", name=f"avB{qs}")
                avD = avp.tile([128, SW], f32, tag="avp", name=f"avD{qs}")
                avAB = [avA, avB]
                nkts = len(kts)
                inter = list(interleave)

                def scores_stage(kt, a, b):
                    ep = []
                    for mh in range(2):
                        pg = scp.tile([128, WW], f32, tag="scp",
                                      name=f"s{qs}_{kt}_{mh}")
                        for hh in range(2):
                            hs = hh * DK
                            nc.tensor.matmul(
                                pg[:KT, hh * SW + a:hh * SW + b],
                                kT[hs:hs + DK, mh * L + kt * KT:mh * L + (kt + 1) * KT],
                                qT[hs:hs + DK, mh * L + qs * SW + a:mh * L + qs * SW + b],
                                start=True, stop=True)
                        et = expp.tile([KT, WW], bf16, tag="exp")
                        if a == 0 and b == SW:
                            nc.scalar.activation(et[:], pg[:KT, :],
                                                 mybir.ActivationFunctionType.Exp)
                        else:
                            for hh in range(2):
                                nc.scalar.activation(
                                    et[:, hh * SW + a:hh * SW + b],
                                    pg[:KT, hh * SW + a:hh * SW + b],
                                    mybir.ActivationFunctionType.Exp)
                        ep.append(et)
                    if cls[qs][kt] == 2:
                        p = pat_idx[qs][kt]
                        for mh in range(2):
                            for hh in range(2):
                                nc.vector.tensor_tensor(
                                    ep[mh][:, hh * SW + a:hh * SW + b],
                                    ep[mh][:, hh * SW + a:hh * SW + b],
                                    mpat_sb[:, p * SW + a:p * SW + b],
                                    mybir.AluOpType.mult)
                    return ep

                def av_stage(i, a, b, kt, ep):
                    st, sp = (i == 0), (i == nkts - 1)
                    for mh in range(2):
                        for hh in range(2):
                            h = 2 * mh + hh
                            nc.tensor.matmul(
                                avAB[mh][hh * DK:(hh + 1) * DK, a:b],
                                vaug[:, kt * VW + h * DK:kt * VW + (h + 1) * DK],
                                ep[mh][:, hh * SW + a:hh * SW + b],
                                start=st, stop=sp)
                    for mh in range(2):
                        for hh in range(2):
                            h = 2 * mh + hh
                            nc.tensor.matmul(
                                avD[32 * h:32 * h + 1, a:b],
                                ones_col[:],
                                ep[mh][:, hh * SW + a:hh * SW + b],
                                start=st, stop=sp,
                                tile_position=(0, 32 * h))

                prev = None
                for i, (kt, a, b) in enumerate(kts):
                    ep = scores_stage(kt, a, b)
                    if inter:
                        inter.pop(0)()
                    if prev is not None:
                        av_stage(*prev)
                    prev = (i, a, b, kt, ep)
                av_stage(*prev)
                for th in inter:
                    th()
                # normalization: reciprocal of the 4 den rows, DRAM-bounce
                # stride-0 broadcast DMAs, apply per head-pair
                r97 = sm.tile([97, SW], f32, tag="r97")
                nc.vector.reciprocal(r97[:], avD[0:97, :])
                bcs = [sm.tile([128, SW], f32, tag=f"bcs{mh}", name=f"bcs{mh}_{qs}")
                       for mh in range(2)]
                for mh in range(2):
                    for hh in range(2):
                        h = 2 * mh + hh
                        dsc = dscrp.tile([1, SW], f32, tag="dscr")
                        nc.sync.dma_start(dsc[:], r97[32 * h:32 * h + 1, :])
                        nc.gpsimd.dma_start(
                            bcs[mh][hh * DK:(hh + 1) * DK, :],
                            dsc[:].to_broadcast([DK, SW]))
                for mh in range(2):
                    nc.vector.tensor_tensor(
                        headT[:, mh * WW + s * SW:mh * WW + (s + 1) * SW],
                        avAB[mh][:, :], bcs[mh][:],
                        mybir.AluOpType.mult)

            def alloc_a2a(w):
                # shard j = 128 window-local rows [j*128, (j+1)*128) of this
                # core's batch, all 256 depth. After the 8-rank A2A, core j
                # holds its 128 rows at full depth from BOTH batches.
                a2a_in = dram.tile([NC, DPC, SROWS], bf16, name=f"a2a_in{w}")
                a2a_out = dram.tile([NC, DPC, SROWS], bf16, name=f"a2a_out{w}")
                return a2a_in, a2a_out

            def ship_half(headT, a2a_in, s):
                for j in range(NG * s, NG * (s + 1)):
                    for mh in range(2):
                        nc.gpsimd.dma_start(
                            a2a_in[j][mh * 128:(mh + 1) * 128, :],
                            headT[:, mh * WW + j * SROWS:mh * WW + (j + 1) * SROWS])

            def trigger_a2a(a2a_in, a2a_out):
                nc.gpsimd.collective_compute(
                    "AllToAll", mybir.AluOpType.bypass,
                    replica_groups=[list(range(NC))],
                    ins=[a2a_in.opt()], outs=[a2a_out.opt()])

            def o_proj_rhs(w, a2a_out):
                # rhs cols: [batch0 128 rows | batch1 128 rows] per depth tile
                rhs = rhp.tile([128, 8 * CROWS], bf16, tag="rh", name=f"rhs{w}")
                for dt in range(8):
                    for g in range(2):
                        eng = nc.sync if dt % 2 == 0 else nc.gpsimd
                        eng.dma_start(
                            rhs[:, dt * CROWS + g * SROWS:
                                dt * CROWS + (g + 1) * SROWS],
                            a2a_out[NG * g + dt // 2][(dt % 2) * 128:
                                                      (dt % 2 + 1) * 128, :])
                return rhs

            def o_proj_mt(w, rhs, mt):
                ps = avp.tile([128, SW], f32, tag="avp", name=f"o{w}{mt}")
                for dt in range(8):
                    nc.tensor.matmul(
                        ps[:, :CROWS],
                        wo_sb[:, dt * DM + mt * 128:dt * DM + (mt + 1) * 128],
                        rhs[:, dt * CROWS:(dt + 1) * CROWS],
                        start=(dt == 0), stop=(dt == 7))
                ob = osb.tile([128, CROWS], f32, tag="osb")
                nc.vector.tensor_scalar_add(ob[:], ps[:, :CROWS],
                                            bo_sb[:, mt:mt + 1])
                nc.sync.dma_start(
                    out_t[mt * 128:(mt + 1) * 128, w * CROWS:(w + 1) * CROWS],
                    ob[:])

            # ---- main schedule ----
            load_half(ytb, 0, "y")
            load_weights_late()          # wv + mask patterns after y0
            load_half(xtb, 0, "x")
            load_half(ytb, 1, "y")
            load_half(xtb, 1, "x")
            load_wo()                    # wo not needed until o_proj(0)

            for s in range(2):
                qk_page(0, "y", wk_sb, bk_sb, kT, s)
            for j in range(8):
                v_kt(0, j)
            for s in range(2):
                qk_page(0, "x", wq_sb, bq_sb, qT, s)

            headT0 = htp.tile([128, 2 * WW], bf16, tag="ht", name="headT0")
            a2a_in0, a2a_out0 = alloc_a2a(0)
            attention_qs(0, headT0, interleave=[
                (lambda s=s: qk_page(1, "y", wk_sb, bk_sb, kT, s))
                for s in range(2)])
            ship_half(headT0, a2a_in0, 0)
            attention_qs(1, headT0, interleave=[
                (lambda j=j: v_kt(1, j)) for j in range(8)
            ] + [
                (lambda s=s: qk_page(1, "x", wq_sb, bq_sb, qT, s))
                for s in range(2)])
            ship_half(headT0, a2a_in0, 1)
            trigger_a2a(a2a_in0, a2a_out0)

            headT1 = htp.tile([128, 2 * WW], bf16, tag="ht", name="headT1")
            a2a_in1, a2a_out1 = alloc_a2a(1)
            rhs0 = o_proj_rhs(0, a2a_out0)
            attention_qs(2, headT1)
            ship_half(headT1, a2a_in1, 0)
            attention_qs(3, headT1, interleave=[
                (lambda mt=mt: o_proj_mt(0, rhs0, mt)) for mt in range(8)])
            ship_half(headT1, a2a_in1, 1)
            trigger_a2a(a2a_in1, a2a_out1)

            # keep the PE clock warm during the last collective wait
            rhs1 = o_proj_rhs(1, a2a_out1)
            for i in range(40):
                wps = scp.tile([128, WW], f32, tag="scp", name=f"warm2_{i}")
                nc.tensor.matmul(wps[:, :SW], qT[:, L - 128:L],
                                 qT[:, L - SW:L], start=True, stop=True)
            for mt in range(8):
                o_proj_mt(1, rhs1, mt)

    nc.compile()
    return nc


def kernel(x, y, mask, Wq, bq, Wk, bk, Wv, bv, Wo, bo, _trace=False):
    x = np.asarray(x, np.float32)
    y = np.asarray(y, np.float32)
    cls, span, pat_idx, pats = _classify_blocks(mask)

    key = (x.shape,
           tuple(tuple(c) for c in cls),
           tuple(tuple(s) for s in span),
           tuple(tuple(p) for p in pat_idx),
           pats.tobytes())
    if key not in _CACHE:
        _CACHE[key] = _build(cls, span, pat_idx, pats.shape[0])
    nc = _CACHE[key]

    fac = np.float32(1.0 / np.sqrt(DK))
    xtb = [np.ascontiguousarray(x[g].T).astype(BF16) for g in range(NB)]
    ytb = [np.ascontiguousarray(y[g].T).astype(BF16) for g in range(NB)]
    Wq32 = np.asarray(Wq, np.float32) * fac
    bq32 = np.asarray(bq, np.float32) * fac
    Wk32 = np.asarray(Wk, np.float32)
    bk32 = np.asarray(bk, np.float32)
    Wv32 = np.asarray(Wv, np.float32)
    bv32 = np.asarray(bv, np.float32)
    wo_b = np.asarray(Wo, np.float32).astype(BF16)
    bo32 = np.asarray(bo, np.float32).reshape(DM, 1)

    in_maps = []
    for c in range(NC):
        g, hg = c // NG, c % NG
        d0 = hg * DPC
        in_maps.append({
            "xtb": xtb[g], "ytb": ytb[g],
            "wq": Wq32[:, d0:d0 + DPC].astype(BF16),
            "wk": Wk32[:, d0:d0 + DPC].astype(BF16),
            "wv": Wv32[:, d0:d0 + DPC].astype(BF16),
            "wo": wo_b,
            "bq": np.stack([bq32[d0:d0 + 128], bq32[d0 + 128:d0 + 256]], axis=1),
            "bk": np.stack([bk32[d0:d0 + 128], bk32[d0 + 128:d0 + 256]], axis=1),
            "bv1": bv32[d0:d0 + DPC].reshape(1, DPC).astype(BF16),
            "bo": bo32,
            "mpat": pats,
        })

    res = run_bass_kernel_spmd(nc, in_maps, core_ids=list(range(NC)), trace=_trace)
    out = np.empty((NB, L, DM), np.float32)
    for c in range(NC):
        o = res.results[c]["out_t"]
        for w in range(NWW):
            for g in range(NB):
                out[g, w * WW + c * SROWS:w * WW + (c + 1) * SROWS, :] = \
                    o[:, w * CROWS + g * SROWS:w * CROWS + (g + 1) * SROWS].T
    if _trace:
        kernel.last_results = res
    return out


# revision 26
# speedup vs baseline: 1.0620x; 1.0620x over previous
"""Multi-head attention (N=2, L=2048, 16 heads x 64) on 8 TRN2 NeuronCores.

Sharding: batch x head hybrid. Cores 0-3 take batch 0, cores 4-7 batch 1;
within each 4-core group a core computes 4 heads (256 depth). Attention is
head-parallel; two small AllToAlls (one per 1024-row window, within the
4-core group) switch to sequence-parallel for the output projection,
pipelined so only the last one is exposed at the end.

Orientation: scores are computed transposed ([k, q]) so attention weights
feed the AV matmul as the moving operand. Heads are processed in pairs:
each score psum page holds 2 heads ([128, 2x512]) so exp is one ScalarE
instruction per page for full blocks; AV packs 2 heads per wave via
col-group tiling (out partitions 0-63 / 64-127) and the softmax
denominators of all 4 heads are rank-1 ones matmuls col-packed into one
psum tile (partitions 0/32/64/96).
"""
import sys

sys.path.insert(0, "/opt/trn_rl_repo")

import numpy as np
import ml_dtypes

import concourse.bass as bass
import concourse.bacc as bacc
import concourse.mybir as mybir
import concourse.tile as tile
from concourse.bass_utils import run_bass_kernel_spmd

BF16 = ml_dtypes.bfloat16

DM = 1024      # dmodel
DK = 64        # head dim
H = 16         # heads
NB = 2         # batch
L = 2048       # seq len per batch
NC = 8         # cores
NG = 4         # cores per group (one batch per group)
HPC = 4        # heads per core
DPC = HPC * DK  # depth per core = 256
VW = DPC       # v width per k-tile

SW = 512       # sub-window (qs granularity)
WW = 1024      # window (A2A granularity)
KT = 128       # k tile
NSW = L // SW   # 4 sub-windows
NWW = L // WW   # 2 windows
NKT = L // KT   # 16 k tiles
SROWS = WW // NC  # 128 rows per A2A shard
CROWS = 2 * SROWS  # 256 output rows per (core, window): 128 from each batch

_CACHE = {}


def _classify_blocks(mask):
    """Per (qs, kt): 0=skip, 1=full, 2=partial (+ q-span, pattern)."""
    mask = np.asarray(mask, dtype=bool)
    cls = [[0] * NKT for _ in range(NSW)]
    span = [[None] * NKT for _ in range(NSW)]
    pat_ids = {}
    pats = []
    pat_idx = [[-1] * NKT for _ in range(NSW)]
    for qs in range(NSW):
        for kt in range(NKT):
            sub = mask[qs * SW:(qs + 1) * SW, kt * KT:(kt + 1) * KT]
            rows = np.nonzero(sub.any(axis=1))[0]
            if rows.size == 0:
                cls[qs][kt] = 0
            elif sub.all():
                cls[qs][kt] = 1
                span[qs][kt] = (0, SW)
            else:
                cls[qs][kt] = 2
                span[qs][kt] = (int(rows[0]), int(rows[-1]) + 1)
                pat = np.ascontiguousarray(sub.T).astype(BF16)  # [128 k, SW q]
                key = pat.tobytes()
                if key not in pat_ids:
                    pat_ids[key] = len(pats)
                    pats.append(pat)
                pat_idx[qs][kt] = pat_ids[key]
    # general-mask safety: the first included kt of each sub-window must cover
    # the full 512 columns (its start=True matmul clears PSUM has_written)
    for qs in range(NSW):
        for kt in range(NKT):
            if cls[qs][kt]:
                span[qs][kt] = (0, SW)
                break
    if not pats:
        pats.append(np.ones((KT, SW), dtype=BF16))
    return cls, span, pat_idx, np.stack(pats)


def _build(cls, span, pat_idx, n_pat):
    nc = bacc.Bacc("TRN2", target_bir_lowering=False, debug=False,
                   enable_asserts=False, num_devices=NC)
    f32, bf16 = mybir.dt.float32, mybir.dt.bfloat16

    xtb = nc.dram_tensor("xtb", [DM, L], bf16, kind="ExternalInput")
    ytb = nc.dram_tensor("ytb", [DM, L], bf16, kind="ExternalInput")
    wq = nc.dram_tensor("wq", [DM, DPC], bf16, kind="ExternalInput")
    wk = nc.dram_tensor("wk", [DM, DPC], bf16, kind="ExternalInput")
    wv = nc.dram_tensor("wv", [DM, DPC], bf16, kind="ExternalInput")
    wo = nc.dram_tensor("wo", [DM, DM], bf16, kind="ExternalInput")
    bqd = nc.dram_tensor("bq", [128, 2], f32, kind="ExternalInput")
    bkd = nc.dram_tensor("bk", [128, 2], f32, kind="ExternalInput")
    bv1 = nc.dram_tensor("bv1", [1, DPC], bf16, kind="ExternalInput")
    bod = nc.dram_tensor("bo", [DM, 1], f32, kind="ExternalInput")
    mpat = nc.dram_tensor("mpat", [n_pat, KT, SW], bf16, kind="ExternalInput")
    out_t = nc.dram_tensor("out_t", [DM, NWW * CROWS], f32, kind="ExternalOutput")

    def qs_kts(qs):
        return [(kt, *span[qs][kt]) for kt in range(NKT) if cls[qs][kt]]

    with tile.TileContext(nc) as tc:
        with (
            tc.tile_pool(name="const", bufs=1) as cst,
            tc.tile_pool(name="xy", bufs=16) as xy,
            tc.tile_pool(name="big", bufs=1) as big,
            tc.tile_pool(name="exp", bufs=10) as expp,
            tc.tile_pool(name="sm", bufs=6) as sm,
            tc.tile_pool(name="ht", bufs=2) as htp,
            tc.tile_pool(name="rh", bufs=2) as rhp,
            tc.tile_pool(name="osb", bufs=3) as osb,
            tc.tile_pool(name="scp", bufs=2, space="PSUM") as scp,
            tc.tile_pool(name="avp", bufs=4, space="PSUM") as avp,
            tc.tile_pool(name="dram", bufs=1, space="DRAM") as dram,
            tc.tile_pool(name="dscr", bufs=8, space="DRAM") as dscrp,
        ):
            # ---- constants to SBUF (DMA issue order matters: wq/wk first
            # for warmup + first projections, then the x/y halves stream in
            # load_half below; wv/wo/mpat are queued after the input halves
            # they precede in consumption order) ----
            wq_sb = cst.tile([128, 8 * DPC], bf16)
            wk_sb = cst.tile([128, 8 * DPC], bf16)
            wv_sb = cst.tile([128, 8 * VW], bf16)
            wo_sb = cst.tile([128, 8 * DM], bf16)
            for dt in range(8):
                nc.sync.dma_start(wq_sb[:, dt * DPC:(dt + 1) * DPC], wq[dt * 128:(dt + 1) * 128, :])
                nc.sync.dma_start(wk_sb[:, dt * DPC:(dt + 1) * DPC], wk[dt * 128:(dt + 1) * 128, :])
            bq_sb = cst.tile([128, 2], f32)
            bk_sb = cst.tile([128, 2], f32)
            nc.sync.dma_start(bq_sb[:], bqd[:])
            nc.sync.dma_start(bk_sb[:], bkd[:])
            bv1_sb = cst.tile([1, DPC], bf16)
            nc.sync.dma_start(bv1_sb[:], bv1[:])
            bo_sb = cst.tile([128, 8], f32)
            for mt in range(8):
                nc.sync.dma_start(bo_sb[:, mt:mt + 1], bod[mt * 128:(mt + 1) * 128, :])
            mpat_sb = cst.tile([KT, n_pat * SW], bf16)
            ones_row = cst.tile([1, 128], bf16)
            nc.vector.memset(ones_row[:], 1.0)
            ones_col = cst.tile([128, 1], bf16)
            nc.vector.memset(ones_col[:], 1.0)
            # rank-1 selector rows at partitions {0,32,64,96} (same partitions
            # as the den rows of heads 0-3): even-head rows select out
            # partitions 0-63, odd-head rows 64-127. Used to broadcast each
            # recip row across its head's partitions via a PE matmul.
            sel128 = cst.tile([128, 128], bf16)
            nc.vector.memset(sel128[:], 0.0)
            for h in range(4):
                lo = 0 if h % 2 == 0 else DK
                nc.vector.memset(sel128[32 * h:32 * h + 1, lo:lo + DK], 1.0)

            def load_weights_late():
                for dt in range(8):
                    nc.sync.dma_start(wv_sb[:, dt * VW:(dt + 1) * VW],
                                      wv[dt * 128:(dt + 1) * 128, :])
                for p in range(n_pat):
                    nc.sync.dma_start(mpat_sb[:, p * SW:(p + 1) * SW], mpat[p])

            def load_wo():
                for dt in range(8):
                    nc.sync.dma_start(wo_sb[:, dt * DM:(dt + 1) * DM],
                                      wo[dt * 128:(dt + 1) * 128, :])

            # start-of-kernel barrier: absorbs per-core launch skew while the
            # big input DMAs stream, so the window AllToAlls aren't skewed
            bar_in = dram.tile([1, 8], f32)
            bar_out = dram.tile([1, 8], f32)
            barrier_sb = cst.tile([1, 8], f32, tag="barrier_sb")
            nc.vector.memset(barrier_sb[:], 0.0)
            nc.sync.dma_start(bar_in[:], barrier_sb[:])
            nc.gpsimd.collective_compute(
                "AllReduce", mybir.AluOpType.add,
                replica_groups=[list(range(NC))],
                ins=[bar_in.opt()], outs=[bar_out.opt()])

            # PE warmup while inputs stream (HAM un-throttle)
            for i in range(16):
                wps = scp.tile([128, WW], f32, tag="scp", name=f"warm{i}")
                nc.tensor.matmul(wps[:, :SW], wq_sb[:, 0:128],
                                 wq_sb[:, 0:SW], start=True, stop=True)

            qT = big.tile([128, 2 * L], bf16)   # mh block * L + col
            kT = big.tile([128, 2 * L], bf16)
            vaug = big.tile([128, NKT * VW], bf16)

            xt_tiles = {}

            def load_half(src, hf, tag):
                # 512-col chunks, all dm-tiles' first chunks before second
                # chunks, so the s=0 projection page can start ~2x earlier
                ts = [xy.tile([128, WW], bf16, tag="xy", name=f"{tag}{hf}_{dt}")
                      for dt in range(8)]
                for s in range(2):
                    for dt in range(8):
                        nc.sync.dma_start(
                            ts[dt][:, s * SW:(s + 1) * SW],
                            src[dt * 128:(dt + 1) * 128,
                                hf * WW + s * SW:hf * WW + (s + 1) * SW])
                xt_tiles[tag, hf] = ts

            def qk_page(hf, tag, wsb, bsb, dst, s):
                tiles = xt_tiles[tag, hf]
                pg = scp.tile([128, WW], f32, tag="scp", name=f"p{tag}{hf}{s}")
                for mh in range(2):
                    for dt in range(8):
                        nc.tensor.matmul(
                            pg[:, mh * SW:(mh + 1) * SW],
                            wsb[:, dt * DPC + mh * 128:dt * DPC + (mh + 1) * 128],
                            tiles[dt][:, s * SW:(s + 1) * SW],
                            start=(dt == 0), stop=(dt == 7))
                for mh in range(2):
                    col = mh * L + hf * WW + s * SW
                    nc.vector.tensor_scalar_add(
                        dst[:, col:col + SW],
                        pg[:, mh * SW:(mh + 1) * SW],
                        bsb[:, mh:mh + 1])

            def v_kt(hf, j):
                tiles = xt_tiles["y", hf]
                kti = hf * 8 + j
                psv = avp.tile([128, SW], f32, tag="avp", name=f"v{kti}")
                for dt in range(8):
                    nc.tensor.matmul(psv[:, :VW],
                                     tiles[dt][:, j * KT:(j + 1) * KT],
                                     wv_sb[:, dt * VW:(dt + 1) * VW],
                                     start=(dt == 0), stop=False)
                nc.tensor.matmul(psv[:, :VW], ones_row[:],
                                 bv1_sb[:], start=False, stop=True)
                nc.vector.tensor_copy(vaug[:, kti * VW:(kti + 1) * VW],
                                      psv[:, :VW])

            def attention_qs(qs, headT, interleave=()):
                """Software-pipelined kt loop: scores(kt_i+1) are issued
                before AV/den(kt_i) so the PE never waits on the exp of the
                block it is about to consume. `interleave` items (thunks of
                extra PE work) are injected between kt stages."""
                kts = qs_kts(qs)
                s = qs % 2
                avA = avp.tile([128, SW], f32, tag="avp", name=f"avA{qs}")
                avB = avp.tile([128, SW], f32, tag="avp", name=f"avB{qs}")
                avD = avp.tile([128, SW], f32, tag="avp", name=f"avD{qs}")
                avAB = [avA, avB]
                nkts = len(kts)
                inter = list(interleave)

                def scores_stage(kt, a, b):
                    ep = []
                    for mh in range(2):
                        pg = scp.tile([128, WW], f32, tag="scp",
                                      name=f"s{qs}_{kt}_{mh}")
                        for hh in range(2):
                            hs = hh * DK
                            nc.tensor.matmul(
                                pg[:KT, hh * SW + a:hh * SW + b],
                                kT[hs:hs + DK, mh * L + kt * KT:mh * L + (kt + 1) * KT],
                                qT[hs:hs + DK, mh * L + qs * SW + a:mh * L + qs * SW + b],
                                start=True, stop=True)
                        et = expp.tile([KT, WW], bf16, tag="exp")
                        if a == 0 and b == SW:
                            nc.scalar.activation(et[:], pg[:KT, :],
                                                 mybir.ActivationFunctionType.Exp)
                        else:
                            for hh in range(2):
                                nc.scalar.activation(
                                    et[:, hh * SW + a:hh * SW + b],
                                    pg[:KT, hh * SW + a:hh * SW + b],
                                    mybir.ActivationFunctionType.Exp)
                        ep.append(et)
                    if cls[qs][kt] == 2:
                        p = pat_idx[qs][kt]
                        for mh in range(2):
                            for hh in range(2):
                                nc.vector.tensor_tensor(
                                    ep[mh][:, hh * SW + a:hh * SW + b],
                                    ep[mh][:, hh * SW + a:hh * SW + b],
                                    mpat_sb[:, p * SW + a:p * SW + b],
                                    mybir.AluOpType.mult)
                    return ep

                def av_stage(i, a, b, kt, ep):
                    st, sp = (i == 0), (i == nkts - 1)
                    for mh in range(2):
                        for hh in range(2):
                            h = 2 * mh + hh
                            nc.tensor.matmul(
                                avAB[mh][hh * DK:(hh + 1) * DK, a:b],
                                vaug[:, kt * VW + h * DK:kt * VW + (h + 1) * DK],
                                ep[mh][:, hh * SW + a:hh * SW + b],
                                start=st, stop=sp)
                    for mh in range(2):
                        for hh in range(2):
                            h = 2 * mh + hh
                            nc.tensor.matmul(
                                avD[32 * h:32 * h + 1, a:b],
                                ones_col[:],
                                ep[mh][:, hh * SW + a:hh * SW + b],
                                start=st, stop=sp,
                                tile_position=(0, 32 * h))

                prev = None
                for i, (kt, a, b) in enumerate(kts):
                    ep = scores_stage(kt, a, b)
                    if inter:
                        inter.pop(0)()
                    if prev is not None:
                        av_stage(*prev)
                    prev = (i, a, b, kt, ep)
                av_stage(*prev)
                for th in inter:
                    th()
                # normalization: reciprocal of the 4 den rows, DRAM-bounce
                # stride-0 broadcast DMAs, apply per head-pair
                r97 = sm.tile([97, SW], f32, tag="r97")
                nc.vector.reciprocal(r97[:], avD[0:97, :])
                bcs = [sm.tile([128, SW], f32, tag=f"bcs{mh}", name=f"bcs{mh}_{qs}")
                       for mh in range(2)]
                for mh in range(2):
                    for hh in range(2):
                        h = 2 * mh + hh
                        dsc = dscrp.tile([1, SW], f32, tag="dscr")
                        nc.sync.dma_start(dsc[:], r97[32 * h:32 * h + 1, :])
                        nc.sync.dma_start(
                            bcs[mh][hh * DK:(hh + 1) * DK, :],
                            dsc[:].to_broadcast([DK, SW]))
                for mh in range(2):
                    nc.vector.tensor_tensor(
                        headT[:, mh * WW + s * SW:mh * WW + (s + 1) * SW],
                        avAB[mh][:, :], bcs[mh][:],
                        mybir.AluOpType.mult)

            def alloc_a2a(w):
                # shard j = 128 window-local rows [j*128, (j+1)*128) of this
                # core's batch, all 256 depth. After the 8-rank A2A, core j
                # holds its 128 rows at full depth from BOTH batches.
                a2a_in = dram.tile([NC, DPC, SROWS], bf16, name=f"a2a_in{w}")
                a2a_out = dram.tile([NC, DPC, SROWS], bf16, name=f"a2a_out{w}")
                return a2a_in, a2a_out

            def ship_half(headT, a2a_in, s):
                for j in range(NG * s, NG * (s + 1)):
                    for mh in range(2):
                        nc.sync.dma_start(
                            a2a_in[j][mh * 128:(mh + 1) * 128, :],
                            headT[:, mh * WW + j * SROWS:mh * WW + (j + 1) * SROWS])

            def trigger_a2a(a2a_in, a2a_out):
                nc.gpsimd.collective_compute(
                    "AllToAll", mybir.AluOpType.bypass,
                    replica_groups=[list(range(NC))],
                    ins=[a2a_in.opt()], outs=[a2a_out.opt()])

            def o_proj_rhs(w, a2a_out):
                # rhs cols: [batch0 128 rows | batch1 128 rows] per depth tile
                rhs = rhp.tile([128, 8 * CROWS], bf16, tag="rh", name=f"rhs{w}")
                for dt in range(8):
                    for g in range(2):
                        nc.sync.dma_start(
                            rhs[:, dt * CROWS + g * SROWS:
                                dt * CROWS + (g + 1) * SROWS],
                            a2a_out[NG * g + dt // 2][(dt % 2) * 128:
                                                      (dt % 2 + 1) * 128, :])
                return rhs

            def o_proj_mt(w, rhs, mt):
                ps = avp.tile([128, SW], f32, tag="avp", name=f"o{w}{mt}")
                for dt in range(8):
                    nc.tensor.matmul(
                        ps[:, :CROWS],
                        wo_sb[:, dt * DM + mt * 128:dt * DM + (mt + 1) * 128],
                        rhs[:, dt * CROWS:(dt + 1) * CROWS],
                        start=(dt == 0), stop=(dt == 7))
                ob = osb.tile([128, CROWS], f32, tag="osb")
                nc.vector.tensor_scalar_add(ob[:], ps[:, :CROWS],
                                            bo_sb[:, mt:mt + 1])
                nc.sync.dma_start(
                    out_t[mt * 128:(mt + 1) * 128, w * CROWS:(w + 1) * CROWS],
                    ob[:])

            # ---- main schedule ----
            load_half(ytb, 0, "y")
            load_weights_late()          # wv + mask patterns after y0
            load_half(xtb, 0, "x")
            load_half(ytb, 1, "y")
            load_half(xtb, 1, "x")
            load_wo()                    # wo not needed until o_proj(0)

            for s in range(2):
                qk_page(0, "y", wk_sb, bk_sb, kT, s)
            for j in range(8):
                v_kt(0, j)
            for s in range(2):
                qk_page(0, "x", wq_sb, bq_sb, qT, s)

            headT0 = htp.tile([128, 2 * WW], bf16, tag="ht", name="headT0")
            a2a_in0, a2a_out0 = alloc_a2a(0)
            attention_qs(0, headT0, interleave=[
                (lambda s=s: qk_page(1, "y", wk_sb, bk_sb, kT, s))
                for s in range(2)])
            ship_half(headT0, a2a_in0, 0)
            attention_qs(1, headT0, interleave=[
                (lambda s=s: qk_page(1, "x", wq_sb, bq_sb, qT, s))
                for s in range(2)
            ] + [
                (lambda j=j: v_kt(1, j)) for j in range(8)])
            ship_half(headT0, a2a_in0, 1)
            trigger_a2a(a2a_in0, a2a_out0)

            headT1 = htp.tile([128, 2 * WW], bf16, tag="ht", name="headT1")
            a2a_in1, a2a_out1 = alloc_a2a(1)
            rhs0 = o_proj_rhs(0, a2a_out0)
            attention_qs(2, headT1)
            ship_half(headT1, a2a_in1, 0)
            attention_qs(3, headT1, interleave=[
                (lambda mt=mt: o_proj_mt(0, rhs0, mt)) for mt in range(8)])
            ship_half(headT1, a2a_in1, 1)
            trigger_a2a(a2a_in1, a2a_out1)

            # keep the PE clock warm during the last collective wait
            rhs1 = o_proj_rhs(1, a2a_out1)
            for i in range(24):
                wps = scp.tile([128, WW], f32, tag="scp", name=f"warm2_{i}")
                nc.tensor.matmul(wps[:, :SW], qT[:, L - 128:L],
                                 qT[:, L - SW:L], start=True, stop=True)
            for mt in range(8):
                o_proj_mt(1, rhs1, mt)

    nc.compile()
    return nc


def kernel(x, y, mask, Wq, bq, Wk, bk, Wv, bv, Wo, bo, _trace=False):
    x = np.asarray(x, np.float32)
    y = np.asarray(y, np.float32)
    cls, span, pat_idx, pats = _classify_blocks(mask)

    key = (x.shape,
           tuple(tuple(c) for c in cls),
           tuple(tuple(s) for s in span),
           tuple(tuple(p) for p in pat_idx),
           pats.tobytes())
    if key not in _CACHE:
        _CACHE[key] = _build(cls, span, pat_idx, pats.shape[0])
    nc = _CACHE[key]

    fac = np.float32(1.0 / np.sqrt(DK))
    xtb = [np.ascontiguousarray(x[g].T).astype(BF16) for g in range(NB)]
    ytb = [np.ascontiguousarray(y[g].T).astype(BF16) for g in range(NB)]
    Wq32 = np.asarray(Wq, np.float32) * fac
    bq32 = np.asarray(bq, np.float32) * fac
    Wk32 = np.asarray(Wk, np.float32)
    bk32 = np.asarray(bk, np.float32)
    Wv32 = np.asarray(Wv, np.float32)
    bv32 = np.asarray(bv, np.float32)
    wo_b = np.asarray(Wo, np.float32).astype(BF16)
    bo32 = np.asarray(bo, np.float32).reshape(DM, 1)

    in_maps = []
    for c in range(NC):
        g, hg = c // NG, c % NG
        d0 = hg * DPC
        in_maps.append({
            "xtb": xtb[g], "ytb": ytb[g],
            "wq": Wq32[:, d0:d0 + DPC].astype(BF16),
            "wk": Wk32[:, d0:d0 + DPC].astype(BF16),
            "wv": Wv32[:, d0:d0 + DPC].astype(BF16),
            "wo": wo_b,
            "bq": np.stack([bq32[d0:d0 + 128], bq32[d0 + 128:d0 + 256]], axis=1),
            "bk": np.stack([bk32[d0:d0 + 128], bk32[d0 + 128:d0 + 256]], axis=1),
            "bv1": bv32[d0:d0 + DPC].reshape(1, DPC).astype(BF16),
            "bo": bo32,
            "mpat": pats,
        })

    res = run_bass_kernel_spmd(nc, in_maps, core_ids=list(range(NC)), trace=_trace)
    out = np.empty((NB, L, DM), np.float32)
    for c in range(NC):
        o = res.results[c]["out_t"]
        for w in range(NWW):
            for g in range(NB):
                out[g, w * WW + c * SROWS:w * WW + (c + 1) * SROWS, :] = \
                    o[:, w * CROWS + g * SROWS:w * CROWS + (g + 1) * SROWS].T
    if _trace:
        kernel.last_results = res
    return out
